# revision 6
# baseline (speedup 1.0000x reference)
"""BitLinear forward (ternary groupwise-quantized linear) on 8 Trainium2 NeuronCores.

Computation:  out = x @ ternary_quantize_groupwise(weight).T
  x: [2, 2048, 4096] f32, weight: [4096, 4096] f32, group=128 along in_features.

Sharding (tensor-parallel, per hint): weight rows (out_features) are split
across 8 cores (512 rows each); x is replicated; each core computes its
[4096, 512] output slice; host concatenates along the feature dim.

Device kernel per core:
  - quantize w shard on-chip: per-group absmean scale (f32, matching the
    reference's thresholding exactly up to reduction order), ternary values
    materialized as q * scale rounded to fp16.
  - x is shipped as an fp16 hi/lo pair (x == hi + lo + O(2^-22)); both halves
    are multiplied by the same fp16 quantized weight on the PE array and
    accumulated in the same fp32 PSUM bank, giving near-fp32 accuracy at
    16-bit matmul throughput.
  - x tiles and the quantized weight are transposed on-chip with the DMA
    xbar transpose (contraction dim must sit on SBUF partitions for the PE).
"""

import os
from contextlib import ExitStack

import numpy as np

import concourse.bass as bass
import concourse.bacc as bacc
import concourse.mybir as mybir
import concourse.tile as tile

# Problem shapes (hardcoded per contract; kernel.py must be self-contained).
B, S, DIM_D, DIM_O = 2, 2048, 4096, 4096
T = B * S                 # 4096 tokens
NCORES = 8
O_SHARD = DIM_O // NCORES  # 512 out features per core
P = 128                    # SBUF partitions / PE array dim
GROUP = 128                # quant group size along in_features
EPS = 1e-8
THRESHOLD = 0.5

f32 = mybir.dt.float32
f16 = mybir.dt.float16


def _emit(ctx, tc, xh, xl, w, out, T_, D_, O_):
    """Emit the per-core program. xh/xl: [T_, D_] f16 DRAM; w: [O_, D_] f32;
    out: [T_, O_] f32."""
    nc = tc.nc
    G = D_ // P            # number of d-chunks == quant groups along D
    OT = O_ // P           # o-tiles of the weight shard
    TT = T_ // P           # token tiles
    NBLK = min(O_, 512)    # psum free dim (one bank at 512 f32)
    NB = O_ // NBLK
    dual = xl is not None

    wpool = ctx.enter_context(tc.tile_pool(name="wnat", bufs=2))
    qpool = ctx.enter_context(tc.tile_pool(name="quant", bufs=1))
    spool = ctx.enter_context(tc.tile_pool(name="stats", bufs=2))
    wqT_pool = ctx.enter_context(tc.tile_pool(name="wqT", bufs=1))
    xT_pool = ctx.enter_context(tc.tile_pool(name="xT", bufs=3))
    opool = ctx.enter_context(tc.tile_pool(name="osb", bufs=3))
    psum = ctx.enter_context(tc.tile_pool(name="psum", bufs=4, space="PSUM"))

    # ---- Phase 1: quantize weight shard, produce wqT [d: P x G, o: O_] f16
    wqT = wqT_pool.tile([P, G, O_], f16, tag="wqT")
    for ot in range(OT):
        wt = wpool.tile([P, D_], f32, tag="wnat")
        nc.scalar.dma_start(wt[:], w[ot * P:(ot + 1) * P, :])
        wg = wt[:].rearrange("p (g j) -> p g j", j=GROUP)

        red = spool.tile([P, G], f32, tag="red")
        nc.vector.tensor_reduce(
            red[:], wg, axis=mybir.AxisListType.X, op=mybir.AluOpType.add,
            apply_absolute_value=True,
        )
        # thr = 0.5 * max(red/128, EPS) = max(red/256, EPS/2)  (all exact in f32)
        thr = spool.tile([P, G], f32, tag="thr")
        nc.vector.tensor_scalar(
            thr[:], red[:], 1.0 / 256.0, EPS / 2.0,
            op0=mybir.AluOpType.mult, op1=mybir.AluOpType.max,
        )
        nthr = spool.tile([P, G], f32, tag="nthr")
        nc.vector.tensor_scalar(
            nthr[:], red[:], -1.0 / 256.0, -EPS / 2.0,
            op0=mybir.AluOpType.mult, op1=mybir.AluOpType.min,
        )
        # scale rounded to f16 (the only precision loss on the weight side)
        s16 = spool.tile([P, G], f16, tag="s16")
        nc.vector.tensor_scalar(
            s16[:], red[:], 1.0 / 128.0, EPS,
            op0=mybir.AluOpType.mult, op1=mybir.AluOpType.max,
        )

        pos = qpool.tile([P, D_], f16, tag="pos")
        nc.vector.tensor_tensor(
            pos[:].rearrange("p (g j) -> p g j", j=GROUP), wg,
            thr[:].unsqueeze(2).broadcast_to((P, G, GROUP)),
            op=mybir.AluOpType.is_gt,
        )
        neg = qpool.tile([P, D_], f16, tag="neg")
        nc.vector.tensor_tensor(
            neg[:].rearrange("p (g j) -> p g j", j=GROUP), wg,
            nthr[:].unsqueeze(2).broadcast_to((P, G, GROUP)),
            op=mybir.AluOpType.is_lt,
        )
        q = qpool.tile([P, D_], f16, tag="q")
        nc.vector.tensor_tensor(q[:], pos[:], neg[:], op=mybir.AluOpType.subtract)
        wqn = qpool.tile([P, D_], f16, tag="wqn")
        nc.vector.tensor_tensor(
            wqn[:].rearrange("p (g j) -> p g j", j=GROUP),
            q[:].rearrange("p (g j) -> p g j", j=GROUP),
            s16[:].unsqueeze(2).broadcast_to((P, G, GROUP)),
            op=mybir.AluOpType.mult,
        )
        # wqT[p, g, ot*P + b] = wqn[b, g*P + p]  (xbar transpose, SBUF->SBUF)
        nc.scalar.dma_start_transpose(wqT[:, :, ot * P:(ot + 1) * P], wqn[:])

    # ---- Phase 2: stream token tiles: transpose x, matmul, store
    for tt in range(TT):
        xTh = xT_pool.tile([P, G, P], f16, tag="xTh")
        nc.sync.dma_start_transpose(xTh[:], xh[tt * P:(tt + 1) * P, :])
        if dual:
            xTl = xT_pool.tile([P, G, P], f16, tag="xTl")
            nc.sync.dma_start_transpose(xTl[:], xl[tt * P:(tt + 1) * P, :])
        for nb in range(NB):
            osl = slice(nb * NBLK, (nb + 1) * NBLK)
            ps = psum.tile([P, NBLK], f32, tag="ps")
            for g in range(G):
                nc.tensor.matmul(
                    ps[:], lhsT=xTh[:, g, :], rhs=wqT[:, g, osl],
                    start=(g == 0), stop=(g == G - 1 and not dual),
                )
                if dual:
                    nc.tensor.matmul(
                        ps[:], lhsT=xTl[:, g, :], rhs=wqT[:, g, osl],
                        start=False, stop=(g == G - 1),
                    )
            osb = opool.tile([P, NBLK], f32, tag="osb")
            nc.vector.tensor_copy(osb[:], ps[:])
            nc.scalar.dma_start(out[tt * P:(tt + 1) * P, osl], osb[:])


def build_nc(T_=T, D_=DIM_D, O_=O_SHARD, dual=True):
    # Bacc (not raw Bass): its compile() legalizes sync waits (walrus allows
    # at most 1 wait per DMA instruction) and fuses nops.
    nc = bacc.Bacc("TRN2", target_bir_lowering=False, debug=False)
    xh = nc.declare_dram_parameter("xh", [T_, D_], f16, isOutput=False)
    xl = nc.declare_dram_parameter("xl", [T_, D_], f16, isOutput=False) if dual else None
    w = nc.declare_dram_parameter("w", [O_, D_], f32, isOutput=False)
    out = nc.declare_dram_parameter("out", [T_, O_], f32, isOutput=True)
    with tile.TileContext(nc) as tc:
        with ExitStack() as ctx:
            _emit(ctx, tc, xh.ap(), xl.ap() if dual else None, w.ap(), out.ap(),
                  T_, D_, O_)
    nc.compile()
    return nc


_NC_CACHE = {}


def _get_nc(dual=True):
    if dual not in _NC_CACHE:
        _NC_CACHE[dual] = build_nc(dual=dual)
    return _NC_CACHE[dual]


def prepare_inputs(x, weight, dual=True):
    xf = np.ascontiguousarray(np.asarray(x, dtype=np.float32).reshape(T, DIM_D))
    wf = np.ascontiguousarray(np.asarray(weight, dtype=np.float32))
    xh = xf.astype(np.float16)
    in_maps = []
    for c in range(NCORES):
        m = {
            "xh": xh,
            "w": np.ascontiguousarray(wf[c * O_SHARD:(c + 1) * O_SHARD]),
        }
        in_maps.append(m)
    if dual:
        xlo = (xf - xh.astype(np.float32)).astype(np.float16)
        for m in in_maps:
            m["xl"] = xlo
    return in_maps


def run(x, weight, dual=True, trace=False, **kwargs):
    from concourse.bass_utils import run_bass_kernel_spmd

    nc = _get_nc(dual=dual)
    in_maps = prepare_inputs(x, weight, dual=dual)
    res = run_bass_kernel_spmd(
        nc, in_maps, core_ids=list(range(NCORES)), trace=trace, **kwargs
    )
    outs = [np.asarray(res.results[c]["out"]) for c in range(NCORES)]
    full = np.concatenate(outs, axis=1).reshape(B, S, DIM_O)
    return full, res


def kernel(x, weight):
    full, _ = run(x, weight, dual=True, trace=False)
    return full.astype(np.float32)


# revision 22
# speedup vs baseline: 1.1160x; 1.1160x over previous
"""BitLinear forward (ternary groupwise-quantized linear) on 8 Trainium2 NeuronCores.

Computation:  out = x @ ternary_quantize_groupwise(weight).T
  x: [2, 2048, 4096] f32, weight: [4096, 4096] f32, group=128 along in_features.

Sharding (tensor-parallel, per hint): weight rows (out_features) are split
across 8 cores (512 rows each); x is replicated; each core computes its
[4096, 512] output slice; host concatenates along the feature dim.

Device kernel per core:
  - quantize w shard on-chip: per-group absmean scale (f32, matching the
    reference's thresholding exactly up to reduction order), ternary values
    materialized as q * scale rounded to fp16.
  - x is shipped as an fp16 hi/lo pair (x == hi + lo + O(2^-22)); both halves
    are multiplied by the same fp16 quantized weight on the PE array and
    accumulated in the same fp32 PSUM bank, giving near-fp32 accuracy at
    16-bit matmul throughput.
  - x tiles and the quantized weight are transposed on-chip with the DMA
    xbar transpose (contraction dim must sit on SBUF partitions for the PE).
"""

import os
from contextlib import ExitStack

import numpy as np

import concourse.bass as bass
import concourse.bacc as bacc
import concourse.mybir as mybir
import concourse.tile as tile

# Problem shapes (hardcoded per contract; kernel.py must be self-contained).
B, S, DIM_D, DIM_O = 2, 2048, 4096, 4096
T = B * S                 # 4096 tokens
NCORES = 8
O_SHARD = DIM_O // NCORES  # 512 out features per core
P = 128                    # SBUF partitions / PE array dim
GROUP = 128                # quant group size along in_features
EPS = 1e-8
THRESHOLD = 0.5

f32 = mybir.dt.float32
f16 = mybir.dt.float16
bf16 = mybir.dt.bfloat16


DEFAULT_CFG = dict(
    # x_hi in bf16: its f16 residual straddles the f16 subnormal range; the
    # bf16 residual (~2^-9 |x|) stays comfortably normal in f16.
    xh_dtype="bfloat16",
    # ALL x transposes must share the sync ring: concurrent DMA-transposes
    # issued from both HWDGE rings corrupt data on HW (measured).
    xl_ring="sync",
    evac="scalar",        # ACT sits closer to PSUM; frees DVE
    store_ring="scalar",  # plain DMAs coexist fine with transposes elsewhere
    w_load="gpsimd",      # SWDGE: own queue, keeps both HWDGE rings free
    reduce_engine="act",  # groupwise abs-sum as 32 ACT accum calls per o-tile
    psum_bufs=6,
    xT_bufs=2,            # buffers of [P, G, T_SPAN] per half
    t_span=256,           # tokens per x transpose: 512B xbar write chunks
)


def _emit(ctx, tc, xh, xl, w, out, T_, D_, O_, cfg):
    """Emit the per-core program. xh/xl: [T_, D_] f16 DRAM; w: [O_, D_] f32;
    out: [T_, O_] f32."""
    nc = tc.nc
    xl_eng = getattr(nc, cfg["xl_ring"])
    store_eng = getattr(nc, cfg["store_ring"])
    G = D_ // P            # number of d-chunks == quant groups along D
    OT = O_ // P           # o-tiles of the weight shard
    TT = T_ // P           # token tiles
    NBLK = min(O_, 512)    # psum free dim (one bank at 512 f32)
    NB = O_ // NBLK
    dual = xl is not None

    wpool = ctx.enter_context(tc.tile_pool(name="wnat", bufs=2))
    qpool = ctx.enter_context(tc.tile_pool(name="quant", bufs=1))
    spool = ctx.enter_context(tc.tile_pool(name="stats", bufs=2))
    wqT_pool = ctx.enter_context(tc.tile_pool(name="wqT", bufs=1))
    xT_pool = ctx.enter_context(tc.tile_pool(name="xT", bufs=cfg["xT_bufs"]))
    opool = ctx.enter_context(tc.tile_pool(name="osb", bufs=4))
    psum = ctx.enter_context(
        tc.tile_pool(name="psum", bufs=cfg["psum_bufs"], space="PSUM"))

    # ---- Phase 1: quantize weight shard, produce wqT [d: P x G, o: O_] f16
    wqT = wqT_pool.tile([P, G, O_], f16, tag="wqT")
    for ot in range(OT):
        wt = wpool.tile([P, D_], f32, tag="wnat")
        getattr(nc, cfg["w_load"]).dma_start(wt[:], w[ot * P:(ot + 1) * P, :])
        wg = wt[:].rearrange("p (g j) -> p g j", j=GROUP)

        red = spool.tile([P, G], f32, tag="red")
        if cfg["reduce_engine"] == "act":
            # groupwise abs-sum on ACT (frees DVE, the quant critical path)
            for g in range(G):
                scr = spool.tile([P, GROUP], f32, tag="scr")
                nc.scalar.activation(
                    scr[:], wt[:, g * GROUP:(g + 1) * GROUP],
                    mybir.ActivationFunctionType.Abs,
                    accum_out=red[:, g:g + 1],
                )
        else:
            nc.vector.tensor_reduce(
                red[:], wg, axis=mybir.AxisListType.X, op=mybir.AluOpType.add,
                apply_absolute_value=True,
            )
        # thr = 0.5 * max(red/128, EPS) = max(red/256, EPS/2)  (all exact in f32)
        thr = spool.tile([P, G], f32, tag="thr")
        nc.vector.tensor_scalar(
            thr[:], red[:], 1.0 / 256.0, EPS / 2.0,
            op0=mybir.AluOpType.mult, op1=mybir.AluOpType.max,
        )
        nthr = spool.tile([P, G], f32, tag="nthr")
        nc.vector.tensor_scalar(
            nthr[:], red[:], -1.0 / 256.0, -EPS / 2.0,
            op0=mybir.AluOpType.mult, op1=mybir.AluOpType.min,
        )
        # scale rounded to f16 (the only precision loss on the weight side)
        s16 = spool.tile([P, G], f16, tag="s16")
        nc.vector.tensor_scalar(
            s16[:], red[:], 1.0 / 128.0, EPS,
            op0=mybir.AluOpType.mult, op1=mybir.AluOpType.max,
        )

        pos = qpool.tile([P, D_], f16, tag="pos")
        nc.vector.tensor_tensor(
            pos[:].rearrange("p (g j) -> p g j", j=GROUP), wg,
            thr[:].unsqueeze(2).broadcast_to((P, G, GROUP)),
            op=mybir.AluOpType.is_gt,
        )
        neg = qpool.tile([P, D_], f16, tag="neg")
        nc.vector.tensor_tensor(
            neg[:].rearrange("p (g j) -> p g j", j=GROUP), wg,
            nthr[:].unsqueeze(2).broadcast_to((P, G, GROUP)),
            op=mybir.AluOpType.is_lt,
        )
        q = qpool.tile([P, D_], f16, tag="q")
        nc.vector.tensor_tensor(q[:], pos[:], neg[:], op=mybir.AluOpType.subtract)
        wqn = qpool.tile([P, D_], f16, tag="wqn")
        nc.vector.tensor_tensor(
            wqn[:].rearrange("p (g j) -> p g j", j=GROUP),
            q[:].rearrange("p (g j) -> p g j", j=GROUP),
            s16[:].unsqueeze(2).broadcast_to((P, G, GROUP)),
            op=mybir.AluOpType.mult,
        )
        # wqT[p, g, ot*P + b] = wqn[b, g*P + p]  (xbar transpose, SBUF->SBUF)
        nc.scalar.dma_start_transpose(wqT[:, :, ot * P:(ot + 1) * P], wqn[:])

    # ---- Phase 2: stream token spans: transpose x, matmul, store
    TSPAN = min(cfg["t_span"], T_)
    SPANS = T_ // TSPAN
    PER = TSPAN // P
    xh_dt = getattr(mybir.dt, cfg["xh_dtype"])
    for s in range(SPANS):
        xTh = xT_pool.tile([P, G, TSPAN], xh_dt, tag="xTh")
        nc.sync.dma_start_transpose(xTh[:], xh[s * TSPAN:(s + 1) * TSPAN, :])
        if dual:
            xTl = xT_pool.tile([P, G, TSPAN], f16, tag="xTl")
            xl_eng.dma_start_transpose(xTl[:], xl[s * TSPAN:(s + 1) * TSPAN, :])
        for sub in range(PER):
            tt = s * PER + sub
            tsl = slice(sub * P, (sub + 1) * P)
            for nb in range(NB):
                osl = slice(nb * NBLK, (nb + 1) * NBLK)
                ps = psum.tile([P, NBLK], f32, tag="ps")
                for g in range(G):
                    nc.tensor.matmul(
                        ps[:], lhsT=xTh[:, g, tsl], rhs=wqT[:, g, osl],
                        start=(g == 0), stop=(g == G - 1 and not dual),
                    )
                    if dual:
                        nc.tensor.matmul(
                            ps[:], lhsT=xTl[:, g, tsl], rhs=wqT[:, g, osl],
                            start=False, stop=(g == G - 1),
                        )
                osb = opool.tile([P, NBLK], f32, tag="osb")
                if cfg["evac"] == "vector":
                    nc.vector.tensor_copy(osb[:], ps[:])
                else:
                    nc.scalar.copy(osb[:], ps[:])
                store_eng.dma_start(out[tt * P:(tt + 1) * P, osl], osb[:])


def build_nc(T_=T, D_=DIM_D, O_=O_SHARD, dual=True, cfg=None):
    cfg = {**DEFAULT_CFG, **(cfg or {})}
    # Bacc (not raw Bass): its compile() legalizes sync waits (walrus allows
    # at most 1 wait per DMA instruction) and fuses nops.
    nc = bacc.Bacc("TRN2", target_bir_lowering=False, debug=False)
    xh_dt = getattr(mybir.dt, cfg["xh_dtype"])
    xh = nc.declare_dram_parameter("xh", [T_, D_], xh_dt, isOutput=False)
    xl = nc.declare_dram_parameter("xl", [T_, D_], f16, isOutput=False) if dual else None
    w = nc.declare_dram_parameter("w", [O_, D_], f32, isOutput=False)
    out = nc.declare_dram_parameter("out", [T_, O_], f32, isOutput=True)
    with tile.TileContext(nc) as tc:
        with ExitStack() as ctx:
            _emit(ctx, tc, xh.ap(), xl.ap() if dual else None, w.ap(), out.ap(),
                  T_, D_, O_, cfg)
    nc.compile()
    return nc


def prepare_inputs(x, weight, dual=True, cfg=None):
    import ml_dtypes

    cfg = {**DEFAULT_CFG, **(cfg or {})}
    xh_np = (ml_dtypes.bfloat16 if cfg["xh_dtype"] == "bfloat16" else np.float16)
    xf = np.ascontiguousarray(np.asarray(x, dtype=np.float32).reshape(T, DIM_D))
    wf = np.ascontiguousarray(np.asarray(weight, dtype=np.float32))
    xh = xf.astype(xh_np)
    in_maps = []
    for c in range(NCORES):
        m = {
            "xh": xh,
            "w": np.ascontiguousarray(wf[c * O_SHARD:(c + 1) * O_SHARD]),
        }
        in_maps.append(m)
    if dual:
        xlo = (xf - xh.astype(np.float32)).astype(np.float16)
        for m in in_maps:
            m["xl"] = xlo
    return in_maps


def run(x, weight, dual=True, trace=False, cfg=None, **kwargs):
    from concourse.bass_utils import run_bass_kernel_spmd

    nc = build_nc(dual=dual, cfg=cfg)
    in_maps = prepare_inputs(x, weight, dual=dual, cfg=cfg)
    res = run_bass_kernel_spmd(
        nc, in_maps, core_ids=list(range(NCORES)), trace=trace, **kwargs
    )
    outs = [np.asarray(res.results[c]["out"]) for c in range(NCORES)]
    full = np.concatenate(outs, axis=1).reshape(B, S, DIM_O)
    return full, res


def kernel(x, weight):
    full, _ = run(x, weight, dual=True, trace=False)
    return full.astype(np.float32)
